# revision 13
# baseline (speedup 1.0000x reference)
"""Multihead attention (B=2, S=2048, D=1024, 16 heads) on 8 trn2 NeuronCores.

Sharding: data-parallel over batch (2 groups of 4 cores), tensor-parallel over
heads within a group (4 heads/core, W_q/W_k/W_v column-sliced, W_o row-sliced).
Each core returns a partial [2048, 1024] bf16 output; the host sums the 4
partials per batch (f64) and adds the constant row bv @ Wo + bo.

v2 changes over the baseline kernel (HW-measured motivations):
- Flipped PV dataflow: P (exp of scores, [kv, q] layout) is the matmul
  STATIONARY and V (+mask column) the moving operand, accumulating
  [128 q, 65] tiles over kv tiles.  Stationary loads overlap the short
  streams on HW (measured 25ns per step), cutting PV from ~27us to ~14.5us
  of PE time, and landing the exp-sums as a per-q COLUMN.
- Normalization via per-partition scalars: reciprocal of the sum column
  [128, 2] + tensor_scalar multiply -> no cross-partition broadcast chain
  (drops the rec0 DMA + gpsimd partition_broadcast of v1).
- The normalized [q, c] tile is bridged to the out-projection's [c, q]
  stationary layout with a PE identity-transpose (+ DVE copy-out); the DMA
  XBAR path costs ~1.9us of ring-sequencer time per 128x128 tile and chokes
  the sync ring.
- W_o and the output partials are bf16 (halves weight + output DMA traffic;
  host accumulates partials in f64).
- K projection for the second head-pair (mh=1) runs as iteration-0 filler
  tasks; phase 1 only computes Q(qc0) + K(mh0), and the kvT DMAs are split
  at kv=512 so K(mh0, chunk0) can start as soon as the first half lands.
  First exp fires ~7us into the kernel instead of ~16us.
"""

import math
import numpy as np

import concourse.bacc as bacc
import concourse.tile as tile
import concourse.mybir as mybir
from concourse.bass_utils import run_bass_kernel_spmd

F32 = mybir.dt.float32
BF16 = mybir.dt.bfloat16
NP_BF16 = mybir.dt.np(BF16)
EXP = mybir.ActivationFunctionType.Exp
MULT = mybir.AluOpType.mult

B, SQ, SKV = 2, 2048, 2048
D, NH, HD = 1024, 16, 64
NCORES = 8
HPC = NH // (NCORES // B)     # 4 heads per core
CS = HPC * HD                 # 256 projection columns per core
QC = 512                      # q chunk
NQC = SQ // QC                # 4 q chunks
NQB = QC // 128               # 4 q blocks per chunk
NDT = D // 128                # 8 contraction tiles

_SKV_P = None                 # packed kv length (multiple of 128), set on host


def _kchunks(skv_p):
    out = []
    off = 0
    while off < skv_p:
        c = min(512, skv_p - off)
        out.append((off, c))
        off += c
    return out


def _build(loop_n: int = 1, skv_p: int | None = None, variant: str = "full"):
    skv_p = skv_p if skv_p is not None else _SKV_P
    assert skv_p is not None and skv_p % 128 == 0
    nkt = skv_p // 128
    nc = bacc.Bacc(None, target_bir_lowering=False)
    xT = nc.dram_tensor("xT", [D, SQ], BF16, kind="ExternalInput")
    kvT = nc.dram_tensor("kvT", [D, skv_p], BF16, kind="ExternalInput")
    wq = nc.dram_tensor("wq", [128, NDT, CS], BF16, kind="ExternalInput")
    wk = nc.dram_tensor("wk", [128, NDT, CS], BF16, kind="ExternalInput")
    wv = nc.dram_tensor("wv", [128, NDT, CS], BF16, kind="ExternalInput")
    wo = nc.dram_tensor("wo", [128, 2, D], BF16, kind="ExternalInput")
    bqk = nc.dram_tensor("bqk", [128, 4], F32, kind="ExternalInput")
    mcol = nc.dram_tensor("mcol", [128, nkt], F32, kind="ExternalInput")
    ident = nc.dram_tensor("ident", [128, 128], BF16, kind="ExternalInput")
    out_p = nc.dram_tensor("out_p", [SQ, D], BF16, kind="ExternalOutput")

    kch = _kchunks(skv_p)
    kv0 = min(512, skv_p)

    with tile.TileContext(nc) as tc:
        with tc.tile_pool(name="const", bufs=1) as const, \
             tc.tile_pool(name="big", bufs=1) as big:
            wq_sb = const.tile([128, NDT, CS], BF16)
            wk_sb = const.tile([128, NDT, CS], BF16)
            wv_sb = const.tile([128, NDT, CS], BF16)
            wo_sb = const.tile([128, 2, D], BF16)
            bqk_sb = const.tile([128, 4], F32)
            mcol_sb = const.tile([128, nkt], F32)
            ident_sb = const.tile([128, 128], BF16)
            ones4 = const.tile([128, HPC, 1], BF16)
            # ring layout: scalar = wk + kvT first halves (done before exps
            # start, keeping the ACT sequencer free for the attention phase);
            # sync = everything latency-critical for Q0; gpsimd (SWDGE) =
            # the rest.
            nc.gpsimd.dma_start(out=wq_sb, in_=wq[:, :, :])
            nc.scalar.dma_start(out=wk_sb, in_=wk[:, :, :])
            nc.sync.dma_start(out=bqk_sb, in_=bqk[:, :])
            nc.vector.memset(ones4, 1.0)

            QTs = {}
            for mh in range(2):
                for qc in range(NQC):
                    QTs[(mh, qc)] = big.tile([128, QC], BF16, tag=f"QT{mh}{qc}",
                                             name=f"QT{mh}{qc}")
            KT = big.tile([128, 2, skv_p], BF16)      # [hd(2x64), mh, kv]
            V = big.tile([128, nkt, HPC, HD + 1], BF16)  # V rows + mask col
            OT = big.tile([128, 2, SQ], BF16)         # [c(2x128), ct, q]

            if loop_n > 1:
                loop_cm = tc.For_i(0, loop_n, 1,
                                   hint_engines=(mybir.EngineType.PE,))
                loop_cm.__enter__()

            # ---- Phase 1: Q(qc0) + K(mh0); K(mh1)/V/Q(qc1-3) become
            # attention fillers ----
            xin_cm = tc.tile_pool(name="xin", bufs=1)
            xin = xin_cm.__enter__()
            kvin_cm = tc.tile_pool(name="kvin", bufs=1)
            kvin = kvin_cm.__enter__()
            if True:
                xts = []
                for dt in range(NDT):
                    xt_t = xin.tile([128, SQ], BF16, tag=f"xt{dt}",
                                    name=f"xt{dt}")
                    nc.sync.dma_start(out=xt_t[:, 0:QC],
                                      in_=xT[dt * 128:(dt + 1) * 128, 0:QC])
                    xts.append(xt_t)
                kvts = []
                for dt in range(NDT):
                    kvt_t = kvin.tile([128, skv_p], BF16, tag=f"kv{dt}",
                                      name=f"kvt{dt}")
                    # first kv half on the fast scalar ring (gates K chunk0)
                    nc.scalar.dma_start(out=kvt_t[:, 0:kv0],
                                        in_=kvT[dt * 128:(dt + 1) * 128, 0:kv0])
                    kvts.append(kvt_t)
                nc.gpsimd.dma_start(out=wv_sb, in_=wv[:, :, :])
                if skv_p > kv0:
                    # second halves on the two HW rings (gpsimd SWDGE desc
                    # processing is ~1us each and was gating K chunk1/V-proj)
                    for dt in range(NDT):
                        eng = nc.sync if dt % 2 == 0 else nc.scalar
                        eng.dma_start(out=kvts[dt][:, kv0:skv_p],
                                      in_=kvT[dt * 128:(dt + 1) * 128,
                                              kv0:skv_p])
                nc.gpsimd.dma_start(out=mcol_sb, in_=mcol[:, :])
                nc.gpsimd.dma_start(out=ident_sb, in_=ident[:, :])
                nc.gpsimd.dma_start(out=wo_sb, in_=wo[:, :, :])
                for dt in range(NDT):
                    nc.sync.dma_start(out=xts[dt][:, QC:SQ],
                                      in_=xT[dt * 128:(dt + 1) * 128, QC:SQ])

                with tc.tile_pool(name="pkv", bufs=1, space="PSUM") as pkv:
                    # Q(qc0): 2 banks, starts as soon as the qc0 xT slices land
                    psq0 = [pkv.tile([128, QC], F32, tag=f"q0{mh}",
                                     name=f"psq0{mh}") for mh in range(2)]
                    for dt in range(NDT):
                        for mh in range(2):
                            nc.tensor.matmul(psq0[mh],
                                             wq_sb[:, dt, mh * 128:(mh + 1) * 128],
                                             xts[dt][:, 0:QC],
                                             start=(dt == 0), stop=(dt == NDT - 1))
                    for mh in range(2):
                        nc.vector.tensor_scalar_add(
                            out=QTs[(mh, 0)],
                            in0=psq0[mh], scalar1=bqk_sb[:, mh:mh + 1])

                    # K(mh0) only; chunk order so scores t0.. can chase
                    psk = {ci: pkv.tile([128, cl], F32, tag=f"pk{ci}",
                                        name=f"psk{ci}")
                           for ci, (off, cl) in enumerate(kch)}
                    for ci, (off, cl) in enumerate(kch):
                        for dt in range(NDT):
                            nc.tensor.matmul(psk[ci],
                                             wk_sb[:, dt, 0:128],
                                             kvts[dt][:, off:off + cl],
                                             start=(dt == 0), stop=(dt == NDT - 1))
                        nc.vector.tensor_scalar_add(
                            out=KT[:, 0, off:off + cl],
                            in0=psk[ci], scalar1=bqk_sb[:, 2:3])

            # ---- Phase 2: attention, software-pipelined over (qc, pair) ----
            from contextlib import ExitStack
            with ExitStack() as ph2:
                pp = ph2.enter_context(tc.tile_pool(name="pp", bufs=3))
                small = ph2.enter_context(tc.tile_pool(name="small", bufs=4))
                outp = ph2.enter_context(tc.tile_pool(name="outp", bufs=2))
                psc = ph2.enter_context(
                    tc.tile_pool(name="psc", bufs=2, space="PSUM"))
                pso = ph2.enter_context(
                    tc.tile_pool(name="pso", bufs=2, space="PSUM"))
                pout = ph2.enter_context(
                    tc.tile_pool(name="pout", bufs=2, space="PSUM"))

                # filler tasks are (pe_cost_ns, fn); emission is budgeted so
                # the PE work slotted between two score groups stays under
                # the exp latency and ACT never starves.
                def kproj1_task(ci):
                    # K(mh1) chunk ci as 4 quarter-tasks (2 dt each)
                    off, cl = kch[ci]
                    pb = pout.tile([128, QC], F32, tag="op", name=f"psk1{ci}")
                    tasks = []
                    for d0 in range(0, NDT, 2):
                        def run(pb=pb, off=off, cl=cl, d0=d0):
                            for dt in (d0, d0 + 1):
                                nc.tensor.matmul(
                                    pb[:, 0:cl], wk_sb[:, dt, 128:256],
                                    kvts[dt][:, off:off + cl],
                                    start=(dt == 0), stop=(dt == NDT - 1))
                            if d0 + 2 == NDT:
                                nc.vector.tensor_scalar_add(
                                    out=KT[:, 1, off:off + cl], in0=pb[:, 0:cl],
                                    scalar1=bqk_sb[:, 3:4])
                        tasks.append((cl * 2 * 0.42, run))
                    return tasks

                def vproj_task(t):
                    pb = pout.tile([128, QC], F32, tag="op", name=f"psv{t}")
                    tasks = []
                    for d0 in (0, NDT // 2):
                        def run(pb=pb, t=t, d0=d0):
                            ps = pb[:, 0:CS]
                            for dt in range(d0, d0 + NDT // 2):
                                nc.tensor.matmul(
                                    ps, kvts[dt][:, t * 128:(t + 1) * 128],
                                    wv_sb[:, dt, :],
                                    start=(dt == 0), stop=(dt == NDT - 1))
                            if d0:
                                nc.vector.tensor_scalar(
                                    out=V[:, t, :, 0:HD],
                                    in0=ps.rearrange("p (h d) -> p h d", h=HPC),
                                    scalar1=mcol_sb[:, t:t + 1], scalar2=None,
                                    op0=MULT)
                                nc.vector.tensor_scalar(
                                    out=V[:, t, :, HD:HD + 1], in0=ones4,
                                    scalar1=mcol_sb[:, t:t + 1], scalar2=None,
                                    op0=MULT)
                        tasks.append((430, run))
                    return tasks

                def qproj_task(mh, qc):
                    pb = pout.tile([128, QC], F32, tag="op", name=f"psq{mh}{qc}")
                    tasks = []
                    for d0 in range(0, NDT, 2):
                        def run(pb=pb, mh=mh, qc=qc, d0=d0):
                            for dt in (d0, d0 + 1):
                                nc.tensor.matmul(
                                    pb, wq_sb[:, dt, mh * 128:(mh + 1) * 128],
                                    xts[dt][:, qc * QC:(qc + 1) * QC],
                                    start=(dt == 0), stop=(dt == NDT - 1))
                            if d0 + 2 == NDT:
                                nc.vector.tensor_scalar_add(
                                    out=QTs[(mh, qc)], in0=pb,
                                    scalar1=bqk_sb[:, mh:mh + 1])
                        tasks.append((430, run))
                    return tasks

                IDENT0 = mybir.ActivationFunctionType.Identity

                def out_proj_half(st, nk):
                    def run(st=st, nk=nk):
                        ot_sb = outp.tile([128, QC], BF16, tag=f"osb{nk}",
                                          name="ot_sb")
                        ps = pout.tile([128, QC], F32, tag="op", name="ps_out")
                        for ct in range(2):
                            nc.tensor.matmul(ps,
                                             OT[:, ct, st * 128:(st + 1) * 128],
                                             wo_sb[:, ct, nk * QC:(nk + 1) * QC],
                                             start=(ct == 0), stop=(ct == 1))
                        # copies run on the (partially idle) ACT engine: the
                        # copy frees the pout bank the next out_proj matmul
                        # WAR-waits on, so keeping it off the busy DVE unstalls
                        # the PE sooner.
                        nc.scalar.activation(out=ot_sb, in_=ps, func=IDENT0)
                        nc.sync.dma_start(
                            out=out_p[st * 128:(st + 1) * 128,
                                      nk * QC:(nk + 1) * QC],
                            in_=ot_sb)
                    return (480, run)

                filler = []

                def emit_filler(budget):
                    while filler and budget > 0:
                        c, fn = filler.pop(0)
                        fn()
                        budget -= c

                exp_q = QC if variant != "tiny_exp" else 64

                iters = [(qc, pr) for qc in range(NQC) for pr in range(2)]
                if variant == "phase1":
                    iters = []
                proj_sched = {}
                if iters:
                    t0_tasks = []
                    for ci in range(len(kch)):
                        t0_tasks.extend(kproj1_task(ci))
                    for t in range(nkt):
                        t0_tasks.extend(vproj_task(t))
                    proj_sched[0] = t0_tasks
                    for qcn in (1, 2, 3):
                        qts = []
                        for mh in range(2):
                            qts.extend(qproj_task(mh, qcn))
                        proj_sched[2 * qcn - 1] = (
                            proj_sched.get(2 * qcn - 1, []) + qts)

                for it_idx, (qc, pr) in enumerate(iters):
                    filler = proj_sched.get(it_idx, []) + filler
                    last = (qc, pr) == iters[-1]
                    P = pp.tile([128, 2, nkt, QC], BF16, tag="P", name="P")
                    chase = None
                    for t in range(nkt):
                        ss = psc.tile([128, 2, QC], F32, tag="ss", name="ss")
                        for h in range(2):
                            po = h * 64
                            nc.tensor.matmul(
                                ss[:, h, :],
                                KT[po:po + 64, pr, t * 128:(t + 1) * 128],
                                QTs[(pr, qc)][po:po + 64, :],
                                start=True, stop=True)
                        nc.scalar.activation(out=P[:, :, t, 0:exp_q],
                                             in_=ss[:, :, 0:exp_q],
                                             func=EXP, scale=0.125)
                        if t > 0:
                            emit_filler(1400 if last else 620)
                        use_chase = (last and nkt >= 4
                                     and variant not in ("nopv", "noout"))
                        if use_chase and t == nkt - 3:
                            # tail shrink: drain the previous pair, then
                            # chase this pair's own exps with the PV chains
                            # (qb0/1 accumulators in pso, qb2/3 in the idle
                            # pout banks), catching up t0..t_now first.
                            # only h=0 is chased: interleaving h0/h1 would
                            # hold two open PSUM accumulation groups in one
                            # bank, which corrupts the accumulation. h1 runs
                            # as a closed chain at the start of the tail.
                            emit_filler(10 ** 9)
                            chase = []
                            for qb in range(NQB):
                                pool_, tg = (pso, "po") if qb < 2 else (pout, "op")
                                chase.append(pool_.tile([128, 2, HD + 1], F32,
                                                        tag=tg, name=f"ch{qb}"))
                            for tt in range(nkt - 2):
                                for qb in range(NQB):
                                    nc.tensor.matmul(
                                        chase[qb][:, 0, :],
                                        P[:, 0, tt, qb * 128:(qb + 1) * 128],
                                        V[:, tt, 2 * pr, :],
                                        start=(tt == 0), stop=False)
                        elif use_chase and t > nkt - 3:
                            for qb in range(NQB):
                                nc.tensor.matmul(
                                    chase[qb][:, 0, :],
                                    P[:, 0, t, qb * 128:(qb + 1) * 128],
                                    V[:, t, 2 * pr, :],
                                    start=False, stop=(t == nkt - 1))

                    # enqueue this pair's PV/norm/transpose (runs as filler
                    # during the next iteration; drained at the end if last)
                    if variant == "nopv":
                        continue

                    def make_tasks(P=P, pr=pr, qc=qc):
                        ts = []
                        for qb in range(NQB):
                            po2 = pso.tile([128, 2, HD + 1], F32, tag="po",
                                           name=f"po2_{qb}")
                            qsl = slice(qb * 128, (qb + 1) * 128)

                            def chain_h(h, P=P, pr=pr, po2=po2, qsl=qsl):
                                for t in range(nkt):
                                    nc.tensor.matmul(
                                        po2[:, h, :],
                                        P[:, h, t, qsl],
                                        V[:, t, 2 * pr + h, :],
                                        start=(t == 0), stop=(t == nkt - 1))

                            def norm(po2=po2, pr=pr, qc=qc, qb=qb):
                                rec = small.tile([128, 2], F32, tag="rec",
                                                 name="rec")
                                nc.vector.reciprocal(
                                    out=rec, in_=po2[:, :, HD:HD + 1])
                                ot2 = small.tile([128, 2, HD], BF16, tag="ot2",
                                                 name="ot2")
                                for h in range(2):
                                    nc.vector.tensor_scalar(
                                        out=ot2[:, h, :], in0=po2[:, h, 0:HD],
                                        scalar1=rec[:, h:h + 1], scalar2=None,
                                        op0=MULT)
                                gq = qc * QC + qb * 128
                                tpo = pso.tile([128, 128], BF16, tag="po",
                                               name="tpo")
                                nc.tensor.transpose(tpo, ot2, ident_sb)
                                nc.scalar.activation(
                                    out=OT[:, pr, gq:gq + 128], in_=tpo,
                                    func=IDENT0)

                            ts.append((650, lambda h=0, f=chain_h: f(h)))
                            ts.append((650, lambda h=1, f=chain_h: f(h)))
                            if variant == "noout":
                                continue
                            ts.append((160, norm))
                            if pr == 1:
                                st = qc * NQB + qb
                                ts.append(out_proj_half(st, 0))
                                ts.append(out_proj_half(st, 1))
                        return ts

                    if chase is None:
                        filler.extend(make_tasks())
                        continue

                    # tail: norms for all 4 qb, then out_projs with the idle
                    # ACT engine doing half the PSUM->SBUF copies and the
                    # final DMAs split across both hardware rings.
                    IDENT = mybir.ActivationFunctionType.Identity
                    for qb in range(NQB):
                        po2 = chase[qb]
                        for t in range(nkt):
                            nc.tensor.matmul(
                                po2[:, 1, :],
                                P[:, 1, t, qb * 128:(qb + 1) * 128],
                                V[:, t, 2 * pr + 1, :],
                                start=(t == 0), stop=(t == nkt - 1))
                        rec = small.tile([128, 2], F32, tag="rec", name="rec")
                        nc.vector.reciprocal(out=rec, in_=po2[:, :, HD:HD + 1])
                        ot2 = small.tile([128, 2, HD], BF16, tag="ot2",
                                         name="ot2")
                        for h in range(2):
                            nc.scalar.activation(
                                out=ot2[:, h, :], in_=po2[:, h, 0:HD],
                                func=IDENT0, scale=rec[:, h:h + 1])
                        gq = qc * QC + qb * 128
                        tpo = pso.tile([128, 128], BF16, tag="po", name="tpo")
                        nc.tensor.transpose(tpo, ot2, ident_sb)
                        nc.scalar.activation(out=OT[:, pr, gq:gq + 128],
                                             in_=tpo, func=IDENT0)
                    for qb in range(NQB):
                        st = qc * NQB + qb
                        ot_sb = outp.tile([128, D], BF16, tag="osb0",
                                          name="ot_sb")
                        for nk in range(2):
                            ps = pout.tile([128, QC], F32, tag="op",
                                           name="ps_out")
                            for ct in range(2):
                                nc.tensor.matmul(
                                    ps, OT[:, ct, st * 128:(st + 1) * 128],
                                    wo_sb[:, ct, nk * QC:(nk + 1) * QC],
                                    start=(ct == 0), stop=(ct == 1))
                            osl = ot_sb[:, nk * QC:(nk + 1) * QC]
                            if nk == 0:
                                nc.scalar.activation(out=osl, in_=ps,
                                                     func=IDENT)
                            else:
                                nc.vector.tensor_copy(out=osl, in_=ps)
                            eng = nc.sync if nk == 0 else nc.scalar
                            eng.dma_start(
                                out=out_p[st * 128:(st + 1) * 128,
                                          nk * QC:(nk + 1) * QC],
                                in_=osl)
                # drain remaining work
                emit_filler(10 ** 9)

            kvin_cm.__exit__(None, None, None)
            xin_cm.__exit__(None, None, None)

            if loop_n > 1:
                loop_cm.__exit__(None, None, None)

    nc.compile()
    return nc


_NC = {}


def _get_nc(skv_p):
    if skv_p not in _NC:
        _NC[skv_p] = _build(skv_p=skv_p)
    return _NC[skv_p]


def _shard_inputs(query_input, key_value_input, key_padding_mask,
                  Wq, bq, Wk, bk, Wv, bv, Wo, bo):
    global _SKV_P
    keep = ~np.asarray(key_padding_mask)
    idxs = [np.nonzero(keep[b])[0] for b in range(B)]
    nmax = max(len(ix) for ix in idxs)
    skv_p = max(256, ((nmax + 127) // 128) * 128)
    _SKV_P = skv_p
    nkt = skv_p // 128

    in_maps = []
    for c in range(NCORES):
        b, hg = c // (NCORES // B), c % (NCORES // B)
        cs = slice(hg * CS, (hg + 1) * CS)
        ix = idxs[b]
        n = len(ix)
        kv_p = np.zeros((skv_p, D), np.float32)
        kv_p[:n] = key_value_input[b][ix]
        m01 = np.zeros((skv_p,), np.float32)
        m01[:n] = 1.0
        mcol = np.ascontiguousarray(m01.reshape(nkt, 128).T)  # [128, nkt]
        in_maps.append({
            "xT": np.ascontiguousarray(query_input[b].T).astype(NP_BF16),
            "kvT": np.ascontiguousarray(kv_p.T).astype(NP_BF16),
            "wq": np.ascontiguousarray(
                Wq[:, cs].reshape(NDT, 128, CS).transpose(1, 0, 2)).astype(NP_BF16),
            "wk": np.ascontiguousarray(
                Wk[:, cs].reshape(NDT, 128, CS).transpose(1, 0, 2)).astype(NP_BF16),
            "wv": np.ascontiguousarray(
                Wv[:, cs].reshape(NDT, 128, CS).transpose(1, 0, 2)).astype(NP_BF16),
            "wo": np.ascontiguousarray(
                Wo[cs, :].reshape(2, 128, D).transpose(1, 0, 2)).astype(NP_BF16),
            "bqk": np.ascontiguousarray(
                np.stack([bq[cs][:128], bq[cs][128:],
                          bk[cs][:128], bk[cs][128:]], axis=1)),
            "mcol": mcol,
            "ident": np.eye(128, dtype=np.float32).astype(NP_BF16),
        })
    return in_maps


def kernel(query_input, key_value_input, key_padding_mask,
           Wq, bq, Wk, bk, Wv, bv, Wo, bo):
    query_input = np.asarray(query_input, np.float32)
    key_value_input = np.asarray(key_value_input, np.float32)
    key_padding_mask = np.asarray(key_padding_mask)
    Wq = np.asarray(Wq, np.float32); bq = np.asarray(bq, np.float32)
    Wk = np.asarray(Wk, np.float32); bk = np.asarray(bk, np.float32)
    Wv = np.asarray(Wv, np.float32); bv = np.asarray(bv, np.float32)
    Wo = np.asarray(Wo, np.float32); bo = np.asarray(bo, np.float32)

    in_maps = _shard_inputs(query_input, key_value_input, key_padding_mask,
                            Wq, bq, Wk, bk, Wv, bv, Wo, bo)
    nc = _get_nc(_SKV_P)
    res = run_bass_kernel_spmd(nc, in_maps, core_ids=list(range(NCORES)))

    # unshard: sum the 4 row-parallel partials per batch; V-bias contributes a
    # constant row (softmax rows sum to 1) folded in with bo here.
    const_row = (bv.astype(np.float64) @ Wo.astype(np.float64)) + bo.astype(np.float64)
    gpc = NCORES // B
    out = np.empty((B, SQ, D), np.float32)
    for b in range(B):
        acc = np.zeros((SQ, D), np.float64)
        for hg in range(gpc):
            acc += res.results[b * gpc + hg]["out_p"].astype(np.float64)
        out[b] = (acc + const_row[None, :]).astype(np.float32)
    return out


# revision 14
# speedup vs baseline: 1.1139x; 1.1139x over previous
"""Multihead attention (B=2, S=2048, D=1024, 16 heads) on 8 trn2 NeuronCores.

Sharding: data-parallel over batch (2 groups of 4 cores), tensor-parallel over
heads within a group (4 heads/core, W_q/W_k/W_v column-sliced, W_o row-sliced).
Each core returns a partial [2048, 1024] bf16 output; the host sums the 4
partials per batch (f64) and adds the constant row bv @ Wo + bo.

v2 changes over the baseline kernel (HW-measured motivations):
- Flipped PV dataflow: P (exp of scores, [kv, q] layout) is the matmul
  STATIONARY and V (+mask column) the moving operand, accumulating
  [128 q, 65] tiles over kv tiles.  Stationary loads overlap the short
  streams on HW (measured 25ns per step), cutting PV from ~27us to ~14.5us
  of PE time, and landing the exp-sums as a per-q COLUMN.
- Normalization via per-partition scalars: reciprocal of the sum column
  [128, 2] + tensor_scalar multiply -> no cross-partition broadcast chain
  (drops the rec0 DMA + gpsimd partition_broadcast of v1).
- The normalized [q, c] tile is bridged to the out-projection's [c, q]
  stationary layout with a PE identity-transpose (+ DVE copy-out); the DMA
  XBAR path costs ~1.9us of ring-sequencer time per 128x128 tile and chokes
  the sync ring.
- W_o and the output partials are bf16 (halves weight + output DMA traffic;
  host accumulates partials in f64).
- K projection for the second head-pair (mh=1) runs as iteration-0 filler
  tasks; phase 1 only computes Q(qc0) + K(mh0), and the kvT DMAs are split
  at kv=512 so K(mh0, chunk0) can start as soon as the first half lands.
  First exp fires ~7us into the kernel instead of ~16us.
"""

import math
import numpy as np

import concourse.bacc as bacc
import concourse.tile as tile
import concourse.mybir as mybir
from concourse.bass_utils import run_bass_kernel_spmd

F32 = mybir.dt.float32
BF16 = mybir.dt.bfloat16
NP_BF16 = mybir.dt.np(BF16)
EXP = mybir.ActivationFunctionType.Exp
MULT = mybir.AluOpType.mult

B, SQ, SKV = 2, 2048, 2048
D, NH, HD = 1024, 16, 64
NCORES = 8
HPC = NH // (NCORES // B)     # 4 heads per core
CS = HPC * HD                 # 256 projection columns per core
QC = 512                      # q chunk
NQC = SQ // QC                # 4 q chunks
NQB = QC // 128               # 4 q blocks per chunk
NDT = D // 128                # 8 contraction tiles

_SKV_P = None                 # packed kv length (multiple of 128), set on host


def _kchunks(skv_p):
    out = []
    off = 0
    while off < skv_p:
        c = min(512, skv_p - off)
        out.append((off, c))
        off += c
    return out


def _build(loop_n: int = 1, skv_p: int | None = None, variant: str = "full"):
    skv_p = skv_p if skv_p is not None else _SKV_P
    assert skv_p is not None and skv_p % 128 == 0
    nkt = skv_p // 128
    nc = bacc.Bacc(None, target_bir_lowering=False)
    xT = nc.dram_tensor("xT", [D, SQ], BF16, kind="ExternalInput")
    kvT = nc.dram_tensor("kvT", [D, skv_p], BF16, kind="ExternalInput")
    wq = nc.dram_tensor("wq", [128, NDT, CS], BF16, kind="ExternalInput")
    wk = nc.dram_tensor("wk", [128, NDT, CS], BF16, kind="ExternalInput")
    wv = nc.dram_tensor("wv", [128, NDT, CS], BF16, kind="ExternalInput")
    wo = nc.dram_tensor("wo", [128, 2, D], BF16, kind="ExternalInput")
    bqk = nc.dram_tensor("bqk", [128, 4], F32, kind="ExternalInput")
    mcol = nc.dram_tensor("mcol", [128, nkt], F32, kind="ExternalInput")
    ident = nc.dram_tensor("ident", [128, 128], BF16, kind="ExternalInput")
    out_p = nc.dram_tensor("out_p", [SQ, D], BF16, kind="ExternalOutput")

    kch = _kchunks(skv_p)
    kv0 = min(512, skv_p)

    with tile.TileContext(nc) as tc:
        with tc.tile_pool(name="const", bufs=1) as const, \
             tc.tile_pool(name="big", bufs=1) as big:
            wq_sb = const.tile([128, NDT, CS], BF16)
            wk_sb = const.tile([128, NDT, CS], BF16)
            wv_sb = const.tile([128, NDT, CS], BF16)
            wo_sb = const.tile([128, 2, D], BF16)
            bqk_sb = const.tile([128, 4], F32)
            mcol_sb = const.tile([128, nkt], F32)
            ident_sb = const.tile([128, 128], BF16)
            ones4 = const.tile([128, HPC, 1], BF16)
            # ring layout: scalar = wk + kvT first halves (done before exps
            # start, keeping the ACT sequencer free for the attention phase);
            # sync = everything latency-critical for Q0; gpsimd (SWDGE) =
            # the rest.
            nc.gpsimd.dma_start(out=wq_sb, in_=wq[:, :, :])
            nc.scalar.dma_start(out=wk_sb, in_=wk[:, :, :])
            nc.sync.dma_start(out=bqk_sb, in_=bqk[:, :])
            nc.vector.memset(ones4, 1.0)

            QTs = {}
            for mh in range(2):
                for qc in range(NQC):
                    QTs[(mh, qc)] = big.tile([128, QC], BF16, tag=f"QT{mh}{qc}",
                                             name=f"QT{mh}{qc}")
            KT = big.tile([128, 2, skv_p], BF16)      # [hd(2x64), mh, kv]
            V = big.tile([128, nkt, HPC, HD + 1], BF16)  # V rows + mask col
            OT = big.tile([128, 2, SQ], BF16)         # [c(2x128), ct, q]

            if loop_n > 1:
                loop_cm = tc.For_i(0, loop_n, 1,
                                   hint_engines=(mybir.EngineType.PE,))
                loop_cm.__enter__()

            # ---- Phase 1: Q(qc0) + K(mh0); K(mh1)/V/Q(qc1-3) become
            # attention fillers ----
            xin_cm = tc.tile_pool(name="xin", bufs=1)
            xin = xin_cm.__enter__()
            kvin_cm = tc.tile_pool(name="kvin", bufs=1)
            kvin = kvin_cm.__enter__()
            if True:
                xts = []
                for dt in range(NDT):
                    xt_t = xin.tile([128, SQ], BF16, tag=f"xt{dt}",
                                    name=f"xt{dt}")
                    nc.sync.dma_start(out=xt_t[:, 0:QC],
                                      in_=xT[dt * 128:(dt + 1) * 128, 0:QC])
                    xts.append(xt_t)
                kvts = []
                for dt in range(NDT):
                    kvt_t = kvin.tile([128, skv_p], BF16, tag=f"kv{dt}",
                                      name=f"kvt{dt}")
                    # first kv half on the fast scalar ring (gates K chunk0)
                    nc.scalar.dma_start(out=kvt_t[:, 0:kv0],
                                        in_=kvT[dt * 128:(dt + 1) * 128, 0:kv0])
                    kvts.append(kvt_t)
                nc.gpsimd.dma_start(out=wv_sb, in_=wv[:, :, :])
                if skv_p > kv0:
                    # second halves on the two HW rings (gpsimd SWDGE desc
                    # processing is ~1us each and was gating K chunk1/V-proj)
                    for dt in range(NDT):
                        eng = nc.sync if dt % 2 == 0 else nc.scalar
                        eng.dma_start(out=kvts[dt][:, kv0:skv_p],
                                      in_=kvT[dt * 128:(dt + 1) * 128,
                                              kv0:skv_p])
                nc.gpsimd.dma_start(out=mcol_sb, in_=mcol[:, :])
                nc.gpsimd.dma_start(out=ident_sb, in_=ident[:, :])
                nc.gpsimd.dma_start(out=wo_sb, in_=wo[:, :, :])
                for dt in range(NDT):
                    nc.sync.dma_start(out=xts[dt][:, QC:SQ],
                                      in_=xT[dt * 128:(dt + 1) * 128, QC:SQ])

                with tc.tile_pool(name="pkv", bufs=1, space="PSUM") as pkv:
                    # Q(qc0): 2 banks, starts as soon as the qc0 xT slices land
                    psq0 = [pkv.tile([128, QC], F32, tag=f"q0{mh}",
                                     name=f"psq0{mh}") for mh in range(2)]
                    for dt in range(NDT):
                        for mh in range(2):
                            nc.tensor.matmul(psq0[mh],
                                             wq_sb[:, dt, mh * 128:(mh + 1) * 128],
                                             xts[dt][:, 0:QC],
                                             start=(dt == 0), stop=(dt == NDT - 1))
                    for mh in range(2):
                        nc.vector.tensor_scalar_add(
                            out=QTs[(mh, 0)],
                            in0=psq0[mh], scalar1=bqk_sb[:, mh:mh + 1])

                    # K(mh0) only; chunk order so scores t0.. can chase
                    psk = {ci: pkv.tile([128, cl], F32, tag=f"pk{ci}",
                                        name=f"psk{ci}")
                           for ci, (off, cl) in enumerate(kch)}
                    for ci, (off, cl) in enumerate(kch):
                        for dt in range(NDT):
                            nc.tensor.matmul(psk[ci],
                                             wk_sb[:, dt, 0:128],
                                             kvts[dt][:, off:off + cl],
                                             start=(dt == 0), stop=(dt == NDT - 1))
                        nc.vector.tensor_scalar_add(
                            out=KT[:, 0, off:off + cl],
                            in0=psk[ci], scalar1=bqk_sb[:, 2:3])

            # ---- Phase 2: attention, software-pipelined over (qc, pair) ----
            from contextlib import ExitStack
            with ExitStack() as ph2:
                pp = ph2.enter_context(tc.tile_pool(name="pp", bufs=3))
                small = ph2.enter_context(tc.tile_pool(name="small", bufs=4))
                outp = ph2.enter_context(tc.tile_pool(name="outp", bufs=2))
                psc = ph2.enter_context(
                    tc.tile_pool(name="psc", bufs=2, space="PSUM"))
                pso = ph2.enter_context(
                    tc.tile_pool(name="pso", bufs=2, space="PSUM"))
                pout = ph2.enter_context(
                    tc.tile_pool(name="pout", bufs=2, space="PSUM"))

                # filler tasks are (pe_cost_ns, fn); emission is budgeted so
                # the PE work slotted between two score groups stays under
                # the exp latency and ACT never starves.
                def kproj1_task(ci):
                    # K(mh1) chunk ci as 4 quarter-tasks (2 dt each)
                    off, cl = kch[ci]
                    pb = pout.tile([128, QC], F32, tag="op", name=f"psk1{ci}")
                    tasks = []
                    for d0 in range(0, NDT, 2):
                        def run(pb=pb, off=off, cl=cl, d0=d0):
                            for dt in (d0, d0 + 1):
                                nc.tensor.matmul(
                                    pb[:, 0:cl], wk_sb[:, dt, 128:256],
                                    kvts[dt][:, off:off + cl],
                                    start=(dt == 0), stop=(dt == NDT - 1))
                            if d0 + 2 == NDT:
                                nc.vector.tensor_scalar_add(
                                    out=KT[:, 1, off:off + cl], in0=pb[:, 0:cl],
                                    scalar1=bqk_sb[:, 3:4])
                        tasks.append((cl * 2 * 0.42, run))
                    return tasks

                def vproj_task(t):
                    pb = pout.tile([128, QC], F32, tag="op", name=f"psv{t}")
                    tasks = []
                    for d0 in (0, NDT // 2):
                        def run(pb=pb, t=t, d0=d0):
                            ps = pb[:, 0:CS]
                            for dt in range(d0, d0 + NDT // 2):
                                nc.tensor.matmul(
                                    ps, kvts[dt][:, t * 128:(t + 1) * 128],
                                    wv_sb[:, dt, :],
                                    start=(dt == 0), stop=(dt == NDT - 1))
                            if d0:
                                nc.vector.tensor_scalar(
                                    out=V[:, t, :, 0:HD],
                                    in0=ps.rearrange("p (h d) -> p h d", h=HPC),
                                    scalar1=mcol_sb[:, t:t + 1], scalar2=None,
                                    op0=MULT)
                                nc.vector.tensor_scalar(
                                    out=V[:, t, :, HD:HD + 1], in0=ones4,
                                    scalar1=mcol_sb[:, t:t + 1], scalar2=None,
                                    op0=MULT)
                        tasks.append((430, run))
                    return tasks

                def qproj_task(mh, qc):
                    pb = pout.tile([128, QC], F32, tag="op", name=f"psq{mh}{qc}")
                    tasks = []
                    for d0 in range(0, NDT, 2):
                        def run(pb=pb, mh=mh, qc=qc, d0=d0):
                            for dt in (d0, d0 + 1):
                                nc.tensor.matmul(
                                    pb, wq_sb[:, dt, mh * 128:(mh + 1) * 128],
                                    xts[dt][:, qc * QC:(qc + 1) * QC],
                                    start=(dt == 0), stop=(dt == NDT - 1))
                            if d0 + 2 == NDT:
                                nc.vector.tensor_scalar_add(
                                    out=QTs[(mh, qc)], in0=pb,
                                    scalar1=bqk_sb[:, mh:mh + 1])
                        tasks.append((430, run))
                    return tasks

                IDENT0 = mybir.ActivationFunctionType.Identity

                def out_proj_half(st, nk):
                    def run(st=st, nk=nk):
                        ot_sb = outp.tile([128, QC], BF16, tag=f"osb{nk}",
                                          name="ot_sb")
                        ps = pout.tile([128, QC], F32, tag="op", name="ps_out")
                        for ct in range(2):
                            nc.tensor.matmul(ps,
                                             OT[:, ct, st * 128:(st + 1) * 128],
                                             wo_sb[:, ct, nk * QC:(nk + 1) * QC],
                                             start=(ct == 0), stop=(ct == 1))
                        # copies run on the (partially idle) ACT engine: the
                        # copy frees the pout bank the next out_proj matmul
                        # WAR-waits on, so keeping it off the busy DVE unstalls
                        # the PE sooner.
                        nc.scalar.activation(out=ot_sb, in_=ps, func=IDENT0)
                        nc.sync.dma_start(
                            out=out_p[st * 128:(st + 1) * 128,
                                      nk * QC:(nk + 1) * QC],
                            in_=ot_sb)
                    return (480, run)

                filler = []

                def emit_filler(budget):
                    while filler and budget > 0:
                        c, fn = filler.pop(0)
                        fn()
                        budget -= c

                exp_q = QC if variant != "tiny_exp" else 64

                iters = [(qc, pr) for qc in range(NQC) for pr in range(2)]
                if variant == "phase1":
                    iters = []
                proj_sched = {}
                if iters:
                    t0_tasks = []
                    for ci in range(len(kch)):
                        t0_tasks.extend(kproj1_task(ci))
                    for t in range(nkt):
                        t0_tasks.extend(vproj_task(t))
                    proj_sched[0] = t0_tasks
                    for qcn in (1, 2, 3):
                        qts = []
                        for mh in range(2):
                            qts.extend(qproj_task(mh, qcn))
                        proj_sched[2 * qcn - 1] = (
                            proj_sched.get(2 * qcn - 1, []) + qts)

                for it_idx, (qc, pr) in enumerate(iters):
                    filler = proj_sched.get(it_idx, []) + filler
                    last = (qc, pr) == iters[-1]
                    P = pp.tile([128, 2, nkt, QC], BF16, tag="P", name="P")
                    chase = None
                    for t in range(nkt):
                        ss = psc.tile([128, 2, QC], F32, tag="ss", name="ss")
                        for h in range(2):
                            po = h * 64
                            nc.tensor.matmul(
                                ss[:, h, :],
                                KT[po:po + 64, pr, t * 128:(t + 1) * 128],
                                QTs[(pr, qc)][po:po + 64, :],
                                start=True, stop=True)
                        nc.scalar.activation(out=P[:, :, t, 0:exp_q],
                                             in_=ss[:, :, 0:exp_q],
                                             func=EXP, scale=0.125)
                        if t > 0:
                            emit_filler(1400 if last else 620)
                        use_chase = (last and nkt >= 4
                                     and variant not in ("nopv", "noout"))
                        if use_chase and t == nkt - 3:
                            # tail shrink: drain the previous pair, then
                            # chase this pair's own exps with the PV chains
                            # (qb0/1 accumulators in pso, qb2/3 in the idle
                            # pout banks), catching up t0..t_now first.
                            # only h=0 is chased: interleaving h0/h1 would
                            # hold two open PSUM accumulation groups in one
                            # bank, which corrupts the accumulation. h1 runs
                            # as a closed chain at the start of the tail.
                            emit_filler(10 ** 9)
                            chase = []
                            for qb in range(NQB):
                                pool_, tg = (pso, "po") if qb < 2 else (pout, "op")
                                chase.append(pool_.tile([128, 2, HD + 1], F32,
                                                        tag=tg, name=f"ch{qb}"))
                            for tt in range(nkt - 2):
                                for qb in range(NQB):
                                    nc.tensor.matmul(
                                        chase[qb][:, 0, :],
                                        P[:, 0, tt, qb * 128:(qb + 1) * 128],
                                        V[:, tt, 2 * pr, :],
                                        start=(tt == 0), stop=False)
                        elif use_chase and t > nkt - 3:
                            for qb in range(NQB):
                                nc.tensor.matmul(
                                    chase[qb][:, 0, :],
                                    P[:, 0, t, qb * 128:(qb + 1) * 128],
                                    V[:, t, 2 * pr, :],
                                    start=False, stop=(t == nkt - 1))

                    # enqueue this pair's PV/norm/transpose (runs as filler
                    # during the next iteration; drained at the end if last)
                    if variant == "nopv":
                        continue

                    def make_tasks(P=P, pr=pr, qc=qc):
                        ts = []
                        for qb in range(NQB):
                            po2 = pso.tile([128, 2, HD + 1], F32, tag="po",
                                           name=f"po2_{qb}")
                            qsl = slice(qb * 128, (qb + 1) * 128)

                            def chain_h(h, P=P, pr=pr, po2=po2, qsl=qsl):
                                for t in range(nkt):
                                    nc.tensor.matmul(
                                        po2[:, h, :],
                                        P[:, h, t, qsl],
                                        V[:, t, 2 * pr + h, :],
                                        start=(t == 0), stop=(t == nkt - 1))

                            def norm(po2=po2, pr=pr, qc=qc, qb=qb):
                                rec = small.tile([128, 2], F32, tag="rec",
                                                 name="rec")
                                nc.vector.reciprocal(
                                    out=rec, in_=po2[:, :, HD:HD + 1])
                                ot2 = small.tile([128, 2, HD], BF16, tag="ot2",
                                                 name="ot2")
                                for h in range(2):
                                    nc.vector.tensor_scalar(
                                        out=ot2[:, h, :], in0=po2[:, h, 0:HD],
                                        scalar1=rec[:, h:h + 1], scalar2=None,
                                        op0=MULT)
                                gq = qc * QC + qb * 128
                                tpo = pso.tile([128, 128], BF16, tag="po",
                                               name="tpo")
                                nc.tensor.transpose(tpo, ot2, ident_sb)
                                nc.vector.tensor_copy(
                                    out=OT[:, pr, gq:gq + 128], in_=tpo)

                            ts.append((650, lambda h=0, f=chain_h: f(h)))
                            ts.append((650, lambda h=1, f=chain_h: f(h)))
                            if variant == "noout":
                                continue
                            ts.append((160, norm))
                            if pr == 1:
                                st = qc * NQB + qb
                                ts.append(out_proj_half(st, 0))
                                ts.append(out_proj_half(st, 1))
                        return ts

                    if chase is None:
                        filler.extend(make_tasks())
                        continue

                    # tail: norms for all 4 qb, then out_projs with the idle
                    # ACT engine doing half the PSUM->SBUF copies and the
                    # final DMAs split across both hardware rings.
                    IDENT = mybir.ActivationFunctionType.Identity
                    for qb in range(NQB):
                        po2 = chase[qb]
                        for t in range(nkt):
                            nc.tensor.matmul(
                                po2[:, 1, :],
                                P[:, 1, t, qb * 128:(qb + 1) * 128],
                                V[:, t, 2 * pr + 1, :],
                                start=(t == 0), stop=(t == nkt - 1))
                        rec = small.tile([128, 2], F32, tag="rec", name="rec")
                        nc.vector.reciprocal(out=rec, in_=po2[:, :, HD:HD + 1])
                        ot2 = small.tile([128, 2, HD], BF16, tag="ot2",
                                         name="ot2")
                        for h in range(2):
                            nc.vector.tensor_scalar(
                                out=ot2[:, h, :], in0=po2[:, h, 0:HD],
                                scalar1=rec[:, h:h + 1], scalar2=None, op0=MULT)
                        gq = qc * QC + qb * 128
                        tpo = pso.tile([128, 128], BF16, tag="po", name="tpo")
                        nc.tensor.transpose(tpo, ot2, ident_sb)
                        nc.vector.tensor_copy(out=OT[:, pr, gq:gq + 128],
                                              in_=tpo)
                    for qb in range(NQB):
                        st = qc * NQB + qb
                        ot_sb = outp.tile([128, D], BF16, tag="osb0",
                                          name="ot_sb")
                        for nk in range(2):
                            ps = pout.tile([128, QC], F32, tag="op",
                                           name="ps_out")
                            for ct in range(2):
                                nc.tensor.matmul(
                                    ps, OT[:, ct, st * 128:(st + 1) * 128],
                                    wo_sb[:, ct, nk * QC:(nk + 1) * QC],
                                    start=(ct == 0), stop=(ct == 1))
                            osl = ot_sb[:, nk * QC:(nk + 1) * QC]
                            if nk == 0:
                                nc.scalar.activation(out=osl, in_=ps,
                                                     func=IDENT)
                            else:
                                nc.vector.tensor_copy(out=osl, in_=ps)
                            eng = nc.sync if nk == 0 else nc.scalar
                            eng.dma_start(
                                out=out_p[st * 128:(st + 1) * 128,
                                          nk * QC:(nk + 1) * QC],
                                in_=osl)
                # drain remaining work
                emit_filler(10 ** 9)

            kvin_cm.__exit__(None, None, None)
            xin_cm.__exit__(None, None, None)

            if loop_n > 1:
                loop_cm.__exit__(None, None, None)

    nc.compile()
    return nc


_NC = {}


def _get_nc(skv_p):
    if skv_p not in _NC:
        _NC[skv_p] = _build(skv_p=skv_p)
    return _NC[skv_p]


def _shard_inputs(query_input, key_value_input, key_padding_mask,
                  Wq, bq, Wk, bk, Wv, bv, Wo, bo):
    global _SKV_P
    keep = ~np.asarray(key_padding_mask)
    idxs = [np.nonzero(keep[b])[0] for b in range(B)]
    nmax = max(len(ix) for ix in idxs)
    skv_p = max(256, ((nmax + 127) // 128) * 128)
    _SKV_P = skv_p
    nkt = skv_p // 128

    in_maps = []
    for c in range(NCORES):
        b, hg = c // (NCORES // B), c % (NCORES // B)
        cs = slice(hg * CS, (hg + 1) * CS)
        ix = idxs[b]
        n = len(ix)
        kv_p = np.zeros((skv_p, D), np.float32)
        kv_p[:n] = key_value_input[b][ix]
        m01 = np.zeros((skv_p,), np.float32)
        m01[:n] = 1.0
        mcol = np.ascontiguousarray(m01.reshape(nkt, 128).T)  # [128, nkt]
        in_maps.append({
            "xT": np.ascontiguousarray(query_input[b].T).astype(NP_BF16),
            "kvT": np.ascontiguousarray(kv_p.T).astype(NP_BF16),
            "wq": np.ascontiguousarray(
                Wq[:, cs].reshape(NDT, 128, CS).transpose(1, 0, 2)).astype(NP_BF16),
            "wk": np.ascontiguousarray(
                Wk[:, cs].reshape(NDT, 128, CS).transpose(1, 0, 2)).astype(NP_BF16),
            "wv": np.ascontiguousarray(
                Wv[:, cs].reshape(NDT, 128, CS).transpose(1, 0, 2)).astype(NP_BF16),
            "wo": np.ascontiguousarray(
                Wo[cs, :].reshape(2, 128, D).transpose(1, 0, 2)).astype(NP_BF16),
            "bqk": np.ascontiguousarray(
                np.stack([bq[cs][:128], bq[cs][128:],
                          bk[cs][:128], bk[cs][128:]], axis=1)),
            "mcol": mcol,
            "ident": np.eye(128, dtype=np.float32).astype(NP_BF16),
        })
    return in_maps


def kernel(query_input, key_value_input, key_padding_mask,
           Wq, bq, Wk, bk, Wv, bv, Wo, bo):
    query_input = np.asarray(query_input, np.float32)
    key_value_input = np.asarray(key_value_input, np.float32)
    key_padding_mask = np.asarray(key_padding_mask)
    Wq = np.asarray(Wq, np.float32); bq = np.asarray(bq, np.float32)
    Wk = np.asarray(Wk, np.float32); bk = np.asarray(bk, np.float32)
    Wv = np.asarray(Wv, np.float32); bv = np.asarray(bv, np.float32)
    Wo = np.asarray(Wo, np.float32); bo = np.asarray(bo, np.float32)

    in_maps = _shard_inputs(query_input, key_value_input, key_padding_mask,
                            Wq, bq, Wk, bk, Wv, bv, Wo, bo)
    nc = _get_nc(_SKV_P)
    res = run_bass_kernel_spmd(nc, in_maps, core_ids=list(range(NCORES)))

    # unshard: sum the 4 row-parallel partials per batch; V-bias contributes a
    # constant row (softmax rows sum to 1) folded in with bo here.
    const_row = (bv.astype(np.float64) @ Wo.astype(np.float64)) + bo.astype(np.float64)
    gpc = NCORES // B
    out = np.empty((B, SQ, D), np.float32)
    for b in range(B):
        acc = np.zeros((SQ, D), np.float64)
        for hg in range(gpc):
            acc += res.results[b * gpc + hg]["out_p"].astype(np.float64)
        out[b] = (acc + const_row[None, :]).astype(np.float32)
    return out


# revision 15
# speedup vs baseline: 1.1699x; 1.0502x over previous
"""Multihead attention (B=2, S=2048, D=1024, 16 heads) on 8 trn2 NeuronCores.

Sharding: data-parallel over batch (2 groups of 4 cores), tensor-parallel over
heads within a group (4 heads/core, W_q/W_k/W_v column-sliced, W_o row-sliced).
Each core returns a partial [2048, 1024] bf16 output; the host sums the 4
partials per batch (f64) and adds the constant row bv @ Wo + bo.

v2 changes over the baseline kernel (HW-measured motivations):
- Flipped PV dataflow: P (exp of scores, [kv, q] layout) is the matmul
  STATIONARY and V (+mask column) the moving operand, accumulating
  [128 q, 65] tiles over kv tiles.  Stationary loads overlap the short
  streams on HW (measured 25ns per step), cutting PV from ~27us to ~14.5us
  of PE time, and landing the exp-sums as a per-q COLUMN.
- Normalization via per-partition scalars: reciprocal of the sum column
  [128, 2] + tensor_scalar multiply -> no cross-partition broadcast chain
  (drops the rec0 DMA + gpsimd partition_broadcast of v1).
- The normalized [q, c] tile is bridged to the out-projection's [c, q]
  stationary layout with a PE identity-transpose (+ DVE copy-out); the DMA
  XBAR path costs ~1.9us of ring-sequencer time per 128x128 tile and chokes
  the sync ring.
- W_o and the output partials are bf16 (halves weight + output DMA traffic;
  host accumulates partials in f64).
- K projection for the second head-pair (mh=1) runs as iteration-0 filler
  tasks; phase 1 only computes Q(qc0) + K(mh0), and the kvT DMAs are split
  at kv=512 so K(mh0, chunk0) can start as soon as the first half lands.
  First exp fires ~7us into the kernel instead of ~16us.
"""

import math
import numpy as np

import concourse.bacc as bacc
import concourse.tile as tile
import concourse.mybir as mybir
from concourse.bass_utils import run_bass_kernel_spmd

F32 = mybir.dt.float32
BF16 = mybir.dt.bfloat16
NP_BF16 = mybir.dt.np(BF16)
EXP = mybir.ActivationFunctionType.Exp
MULT = mybir.AluOpType.mult

B, SQ, SKV = 2, 2048, 2048
D, NH, HD = 1024, 16, 64
NCORES = 8
HPC = NH // (NCORES // B)     # 4 heads per core
CS = HPC * HD                 # 256 projection columns per core
QC = 512                      # q chunk
NQC = SQ // QC                # 4 q chunks
NQB = QC // 128               # 4 q blocks per chunk
NDT = D // 128                # 8 contraction tiles

_SKV_P = None                 # packed kv length (multiple of 128), set on host


def _kchunks(skv_p):
    out = []
    off = 0
    while off < skv_p:
        c = min(512, skv_p - off)
        out.append((off, c))
        off += c
    return out


def _build(loop_n: int = 1, skv_p: int | None = None, variant: str = "full"):
    skv_p = skv_p if skv_p is not None else _SKV_P
    assert skv_p is not None and skv_p % 128 == 0
    nkt = skv_p // 128
    nc = bacc.Bacc(None, target_bir_lowering=False)
    xT = nc.dram_tensor("xT", [128, NDT, SQ], BF16, kind="ExternalInput")
    kvT = nc.dram_tensor("kvT", [128, NDT, skv_p], BF16, kind="ExternalInput")
    wq = nc.dram_tensor("wq", [128, NDT, CS], BF16, kind="ExternalInput")
    wk = nc.dram_tensor("wk", [128, NDT, CS], BF16, kind="ExternalInput")
    wv = nc.dram_tensor("wv", [128, NDT, CS], BF16, kind="ExternalInput")
    wo = nc.dram_tensor("wo", [128, 2, D], BF16, kind="ExternalInput")
    bqk = nc.dram_tensor("bqk", [128, 4], F32, kind="ExternalInput")
    mcol = nc.dram_tensor("mcol", [128, nkt], F32, kind="ExternalInput")
    ident = nc.dram_tensor("ident", [128, 128], BF16, kind="ExternalInput")
    out_p = nc.dram_tensor("out_p", [SQ, D], BF16, kind="ExternalOutput")

    kch = _kchunks(skv_p)
    kv0 = min(512, skv_p)

    with tile.TileContext(nc) as tc:
        with tc.tile_pool(name="const", bufs=1) as const, \
             tc.tile_pool(name="big", bufs=1) as big:
            wq_sb = const.tile([128, NDT, CS], BF16)
            wk_sb = const.tile([128, NDT, CS], BF16)
            wv_sb = const.tile([128, NDT, CS], BF16)
            wo_sb = const.tile([128, 2, D], BF16)
            bqk_sb = const.tile([128, 4], F32)
            mcol_sb = const.tile([128, nkt], F32)
            ident_sb = const.tile([128, 128], BF16)
            ones4 = const.tile([128, HPC, 1], BF16)
            # ring layout: scalar = wk + kvT first halves (done before exps
            # start, keeping the ACT sequencer free for the attention phase);
            # sync = everything latency-critical for Q0; gpsimd (SWDGE) =
            # the rest.
            nc.gpsimd.dma_start(out=wq_sb, in_=wq[:, :, :])
            nc.scalar.dma_start(out=wk_sb, in_=wk[:, :, :])
            nc.sync.dma_start(out=bqk_sb, in_=bqk[:, :])
            nc.vector.memset(ones4, 1.0)

            QTs = {}
            for mh in range(2):
                for qc in range(NQC):
                    QTs[(mh, qc)] = big.tile([128, QC], BF16, tag=f"QT{mh}{qc}",
                                             name=f"QT{mh}{qc}")
            KT = big.tile([128, 2, skv_p], BF16)      # [hd(2x64), mh, kv]
            V = big.tile([128, nkt, HPC, HD + 1], BF16)  # V rows + mask col
            OT = big.tile([128, 2, SQ], BF16)         # [c(2x128), ct, q]

            if loop_n > 1:
                loop_cm = tc.For_i(0, loop_n, 1,
                                   hint_engines=(mybir.EngineType.PE,))
                loop_cm.__enter__()

            # ---- Phase 1: Q(qc0) + K(mh0); K(mh1)/V/Q(qc1-3) become
            # attention fillers ----
            xin_cm = tc.tile_pool(name="xin", bufs=1)
            xin = xin_cm.__enter__()
            kvin_cm = tc.tile_pool(name="kvin", bufs=1)
            kvin = kvin_cm.__enter__()
            if True:
                # partition-major xT/kvT: 2 dt-tiles per DMA descriptor
                # halves ring occupancy on both head-critical paths
                xt_all = xin.tile([128, NDT, SQ], BF16, tag="xt", name="xt")
                xts = [xt_all[:, dt, :] for dt in range(NDT)]
                for d0 in range(0, NDT, 2):
                    nc.sync.dma_start(out=xt_all[:, d0:d0 + 2, 0:QC],
                                      in_=xT[:, d0:d0 + 2, 0:QC])
                kvt_all = kvin.tile([128, NDT, skv_p], BF16, tag="kv",
                                    name="kv")
                kvts = [kvt_all[:, dt, :] for dt in range(NDT)]
                for d0 in range(0, NDT, 2):
                    # first kv half on the fast scalar ring (gates K chunk0)
                    nc.scalar.dma_start(out=kvt_all[:, d0:d0 + 2, 0:kv0],
                                        in_=kvT[:, d0:d0 + 2, 0:kv0])
                nc.gpsimd.dma_start(out=wv_sb, in_=wv[:, :, :])
                if skv_p > kv0:
                    for d0 in range(0, NDT, 2):
                        eng = nc.sync if d0 % 4 == 0 else nc.scalar
                        eng.dma_start(out=kvt_all[:, d0:d0 + 2, kv0:skv_p],
                                      in_=kvT[:, d0:d0 + 2, kv0:skv_p])
                nc.gpsimd.dma_start(out=mcol_sb, in_=mcol[:, :])
                nc.gpsimd.dma_start(out=ident_sb, in_=ident[:, :])
                nc.gpsimd.dma_start(out=wo_sb, in_=wo[:, :, :])
                for d0 in range(0, NDT, 2):
                    nc.sync.dma_start(out=xt_all[:, d0:d0 + 2, QC:SQ],
                                      in_=xT[:, d0:d0 + 2, QC:SQ])

                with tc.tile_pool(name="pkv", bufs=1, space="PSUM") as pkv:
                    # Q(qc0): 2 banks, starts as soon as the qc0 xT slices land
                    psq0 = [pkv.tile([128, QC], F32, tag=f"q0{mh}",
                                     name=f"psq0{mh}") for mh in range(2)]
                    for dt in range(NDT):
                        for mh in range(2):
                            nc.tensor.matmul(psq0[mh],
                                             wq_sb[:, dt, mh * 128:(mh + 1) * 128],
                                             xts[dt][:, 0:QC],
                                             start=(dt == 0), stop=(dt == NDT - 1))
                    for mh in range(2):
                        nc.vector.tensor_scalar_add(
                            out=QTs[(mh, 0)],
                            in0=psq0[mh], scalar1=bqk_sb[:, mh:mh + 1])

                    # K(mh0) only; chunk order so scores t0.. can chase
                    psk = {ci: pkv.tile([128, cl], F32, tag=f"pk{ci}",
                                        name=f"psk{ci}")
                           for ci, (off, cl) in enumerate(kch)}
                    for ci, (off, cl) in enumerate(kch):
                        for dt in range(NDT):
                            nc.tensor.matmul(psk[ci],
                                             wk_sb[:, dt, 0:128],
                                             kvts[dt][:, off:off + cl],
                                             start=(dt == 0), stop=(dt == NDT - 1))
                        nc.vector.tensor_scalar_add(
                            out=KT[:, 0, off:off + cl],
                            in0=psk[ci], scalar1=bqk_sb[:, 2:3])

            # ---- Phase 2: attention, software-pipelined over (qc, pair) ----
            from contextlib import ExitStack
            with ExitStack() as ph2:
                pp = ph2.enter_context(tc.tile_pool(name="pp", bufs=3))
                small = ph2.enter_context(tc.tile_pool(name="small", bufs=4))
                outp = ph2.enter_context(tc.tile_pool(name="outp", bufs=2))
                psc = ph2.enter_context(
                    tc.tile_pool(name="psc", bufs=2, space="PSUM"))
                pso = ph2.enter_context(
                    tc.tile_pool(name="pso", bufs=2, space="PSUM"))
                pout = ph2.enter_context(
                    tc.tile_pool(name="pout", bufs=2, space="PSUM"))

                # filler tasks are (pe_cost_ns, fn); emission is budgeted so
                # the PE work slotted between two score groups stays under
                # the exp latency and ACT never starves.
                def kproj1_task(ci):
                    # K(mh1) chunk ci as 4 quarter-tasks (2 dt each)
                    off, cl = kch[ci]
                    pb = pout.tile([128, QC], F32, tag="op", name=f"psk1{ci}")
                    tasks = []
                    for d0 in range(0, NDT, 2):
                        def run(pb=pb, off=off, cl=cl, d0=d0):
                            for dt in (d0, d0 + 1):
                                nc.tensor.matmul(
                                    pb[:, 0:cl], wk_sb[:, dt, 128:256],
                                    kvts[dt][:, off:off + cl],
                                    start=(dt == 0), stop=(dt == NDT - 1))
                            if d0 + 2 == NDT:
                                nc.vector.tensor_scalar_add(
                                    out=KT[:, 1, off:off + cl], in0=pb[:, 0:cl],
                                    scalar1=bqk_sb[:, 3:4])
                        tasks.append((cl * 2 * 0.42, run))
                    return tasks

                def vproj_task(t):
                    pb = pout.tile([128, QC], F32, tag="op", name=f"psv{t}")
                    tasks = []
                    for d0 in (0, NDT // 2):
                        def run(pb=pb, t=t, d0=d0):
                            ps = pb[:, 0:CS]
                            for dt in range(d0, d0 + NDT // 2):
                                nc.tensor.matmul(
                                    ps, kvts[dt][:, t * 128:(t + 1) * 128],
                                    wv_sb[:, dt, :],
                                    start=(dt == 0), stop=(dt == NDT - 1))
                            if d0:
                                nc.vector.tensor_scalar(
                                    out=V[:, t, :, 0:HD],
                                    in0=ps.rearrange("p (h d) -> p h d", h=HPC),
                                    scalar1=mcol_sb[:, t:t + 1], scalar2=None,
                                    op0=MULT)
                                nc.vector.tensor_scalar(
                                    out=V[:, t, :, HD:HD + 1], in0=ones4,
                                    scalar1=mcol_sb[:, t:t + 1], scalar2=None,
                                    op0=MULT)
                        tasks.append((430, run))
                    return tasks

                def qproj_task(mh, qc):
                    pb = pout.tile([128, QC], F32, tag="op", name=f"psq{mh}{qc}")
                    tasks = []
                    for d0 in range(0, NDT, 2):
                        def run(pb=pb, mh=mh, qc=qc, d0=d0):
                            for dt in (d0, d0 + 1):
                                nc.tensor.matmul(
                                    pb, wq_sb[:, dt, mh * 128:(mh + 1) * 128],
                                    xts[dt][:, qc * QC:(qc + 1) * QC],
                                    start=(dt == 0), stop=(dt == NDT - 1))
                            if d0 + 2 == NDT:
                                nc.vector.tensor_scalar_add(
                                    out=QTs[(mh, qc)], in0=pb,
                                    scalar1=bqk_sb[:, mh:mh + 1])
                        tasks.append((430, run))
                    return tasks

                IDENT0 = mybir.ActivationFunctionType.Identity

                def out_proj_half(st, nk):
                    def run(st=st, nk=nk):
                        ot_sb = outp.tile([128, QC], BF16, tag=f"osb{nk}",
                                          name="ot_sb")
                        ps = pout.tile([128, QC], F32, tag="op", name="ps_out")
                        for ct in range(2):
                            nc.tensor.matmul(ps,
                                             OT[:, ct, st * 128:(st + 1) * 128],
                                             wo_sb[:, ct, nk * QC:(nk + 1) * QC],
                                             start=(ct == 0), stop=(ct == 1))
                        # copies run on the (partially idle) ACT engine: the
                        # copy frees the pout bank the next out_proj matmul
                        # WAR-waits on, so keeping it off the busy DVE unstalls
                        # the PE sooner.
                        nc.scalar.activation(out=ot_sb, in_=ps, func=IDENT0)
                        nc.sync.dma_start(
                            out=out_p[st * 128:(st + 1) * 128,
                                      nk * QC:(nk + 1) * QC],
                            in_=ot_sb)
                    return (480, run)

                filler = []

                def emit_filler(budget):
                    while filler and budget > 0:
                        c, fn = filler.pop(0)
                        fn()
                        budget -= c

                exp_q = QC if variant != "tiny_exp" else 64

                iters = [(qc, pr) for qc in range(NQC) for pr in range(2)]
                if variant == "phase1":
                    iters = []
                proj_sched = {}
                if iters:
                    t0_tasks = []
                    for ci in range(len(kch)):
                        t0_tasks.extend(kproj1_task(ci))
                    for t in range(nkt):
                        t0_tasks.extend(vproj_task(t))
                    proj_sched[0] = t0_tasks
                    for qcn in (1, 2, 3):
                        qts = []
                        for mh in range(2):
                            qts.extend(qproj_task(mh, qcn))
                        proj_sched[2 * qcn - 1] = (
                            proj_sched.get(2 * qcn - 1, []) + qts)

                for it_idx, (qc, pr) in enumerate(iters):
                    filler = proj_sched.get(it_idx, []) + filler
                    last = (qc, pr) == iters[-1]
                    P = pp.tile([128, 2, nkt, QC], BF16, tag="P", name="P")
                    chase = None
                    for t in range(nkt):
                        ss = psc.tile([128, 2, QC], F32, tag="ss", name="ss")
                        for h in range(2):
                            po = h * 64
                            nc.tensor.matmul(
                                ss[:, h, :],
                                KT[po:po + 64, pr, t * 128:(t + 1) * 128],
                                QTs[(pr, qc)][po:po + 64, :],
                                start=True, stop=True)
                        nc.scalar.activation(out=P[:, :, t, 0:exp_q],
                                             in_=ss[:, :, 0:exp_q],
                                             func=EXP, scale=0.125)
                        if t > 0:
                            emit_filler(1400 if last else 620)
                        use_chase = (last and nkt >= 4
                                     and variant not in ("nopv", "noout"))
                        if use_chase and t == nkt - 3:
                            # tail shrink: drain the previous pair, then
                            # chase this pair's own exps with the PV chains
                            # (qb0/1 accumulators in pso, qb2/3 in the idle
                            # pout banks), catching up t0..t_now first.
                            # only h=0 is chased: interleaving h0/h1 would
                            # hold two open PSUM accumulation groups in one
                            # bank, which corrupts the accumulation. h1 runs
                            # as a closed chain at the start of the tail.
                            emit_filler(10 ** 9)
                            chase = []
                            for qb in range(NQB):
                                pool_, tg = (pso, "po") if qb < 2 else (pout, "op")
                                chase.append(pool_.tile([128, 2, HD + 1], F32,
                                                        tag=tg, name=f"ch{qb}"))
                            for tt in range(nkt - 2):
                                for qb in range(NQB):
                                    nc.tensor.matmul(
                                        chase[qb][:, 0, :],
                                        P[:, 0, tt, qb * 128:(qb + 1) * 128],
                                        V[:, tt, 2 * pr, :],
                                        start=(tt == 0), stop=False)
                        elif use_chase and t > nkt - 3:
                            for qb in range(NQB):
                                nc.tensor.matmul(
                                    chase[qb][:, 0, :],
                                    P[:, 0, t, qb * 128:(qb + 1) * 128],
                                    V[:, t, 2 * pr, :],
                                    start=False, stop=(t == nkt - 1))

                    # enqueue this pair's PV/norm/transpose (runs as filler
                    # during the next iteration; drained at the end if last)
                    if variant == "nopv":
                        continue

                    def make_tasks(P=P, pr=pr, qc=qc):
                        ts = []
                        for qb in range(NQB):
                            po2 = pso.tile([128, 2, HD + 1], F32, tag="po",
                                           name=f"po2_{qb}")
                            qsl = slice(qb * 128, (qb + 1) * 128)

                            def chain_h(h, P=P, pr=pr, po2=po2, qsl=qsl):
                                for t in range(nkt):
                                    nc.tensor.matmul(
                                        po2[:, h, :],
                                        P[:, h, t, qsl],
                                        V[:, t, 2 * pr + h, :],
                                        start=(t == 0), stop=(t == nkt - 1))

                            def norm(po2=po2, pr=pr, qc=qc, qb=qb):
                                rec = small.tile([128, 2], F32, tag="rec",
                                                 name="rec")
                                nc.vector.reciprocal(
                                    out=rec, in_=po2[:, :, HD:HD + 1])
                                ot2 = small.tile([128, 2, HD], BF16, tag="ot2",
                                                 name="ot2")
                                for h in range(2):
                                    nc.vector.tensor_scalar(
                                        out=ot2[:, h, :], in0=po2[:, h, 0:HD],
                                        scalar1=rec[:, h:h + 1], scalar2=None,
                                        op0=MULT)
                                gq = qc * QC + qb * 128
                                tpo = pso.tile([128, 128], BF16, tag="po",
                                               name="tpo")
                                nc.tensor.transpose(tpo, ot2, ident_sb)
                                nc.vector.tensor_copy(
                                    out=OT[:, pr, gq:gq + 128], in_=tpo)

                            ts.append((650, lambda h=0, f=chain_h: f(h)))
                            ts.append((650, lambda h=1, f=chain_h: f(h)))
                            if variant == "noout":
                                continue
                            ts.append((160, norm))
                            if pr == 1:
                                st = qc * NQB + qb
                                ts.append(out_proj_half(st, 0))
                                ts.append(out_proj_half(st, 1))
                        return ts

                    if chase is None:
                        filler.extend(make_tasks())
                        continue

                    # tail: norms for all 4 qb, then out_projs with the idle
                    # ACT engine doing half the PSUM->SBUF copies and the
                    # final DMAs split across both hardware rings.
                    IDENT = mybir.ActivationFunctionType.Identity
                    for qb in range(NQB):
                        po2 = chase[qb]
                        for t in range(nkt):
                            nc.tensor.matmul(
                                po2[:, 1, :],
                                P[:, 1, t, qb * 128:(qb + 1) * 128],
                                V[:, t, 2 * pr + 1, :],
                                start=(t == 0), stop=(t == nkt - 1))
                        rec = small.tile([128, 2], F32, tag="rec", name="rec")
                        nc.vector.reciprocal(out=rec, in_=po2[:, :, HD:HD + 1])
                        ot2 = small.tile([128, 2, HD], BF16, tag="ot2",
                                         name="ot2")
                        for h in range(2):
                            nc.vector.tensor_scalar(
                                out=ot2[:, h, :], in0=po2[:, h, 0:HD],
                                scalar1=rec[:, h:h + 1], scalar2=None, op0=MULT)
                        gq = qc * QC + qb * 128
                        tpo = pso.tile([128, 128], BF16, tag="po", name="tpo")
                        nc.tensor.transpose(tpo, ot2, ident_sb)
                        nc.vector.tensor_copy(out=OT[:, pr, gq:gq + 128],
                                              in_=tpo)
                    for qb in range(NQB):
                        st = qc * NQB + qb
                        ot_sb = outp.tile([128, D], BF16, tag="osb0",
                                          name="ot_sb")
                        for nk in range(2):
                            ps = pout.tile([128, QC], F32, tag="op",
                                           name="ps_out")
                            for ct in range(2):
                                nc.tensor.matmul(
                                    ps, OT[:, ct, st * 128:(st + 1) * 128],
                                    wo_sb[:, ct, nk * QC:(nk + 1) * QC],
                                    start=(ct == 0), stop=(ct == 1))
                            osl = ot_sb[:, nk * QC:(nk + 1) * QC]
                            if nk == 0:
                                nc.scalar.activation(out=osl, in_=ps,
                                                     func=IDENT)
                            else:
                                nc.vector.tensor_copy(out=osl, in_=ps)
                            eng = nc.sync if nk == 0 else nc.scalar
                            eng.dma_start(
                                out=out_p[st * 128:(st + 1) * 128,
                                          nk * QC:(nk + 1) * QC],
                                in_=osl)
                # drain remaining work
                emit_filler(10 ** 9)

            kvin_cm.__exit__(None, None, None)
            xin_cm.__exit__(None, None, None)

            if loop_n > 1:
                loop_cm.__exit__(None, None, None)

    nc.compile()
    return nc


_NC = {}


def _get_nc(skv_p):
    if skv_p not in _NC:
        _NC[skv_p] = _build(skv_p=skv_p)
    return _NC[skv_p]


def _shard_inputs(query_input, key_value_input, key_padding_mask,
                  Wq, bq, Wk, bk, Wv, bv, Wo, bo):
    global _SKV_P
    keep = ~np.asarray(key_padding_mask)
    idxs = [np.nonzero(keep[b])[0] for b in range(B)]
    nmax = max(len(ix) for ix in idxs)
    skv_p = max(256, ((nmax + 127) // 128) * 128)
    _SKV_P = skv_p
    nkt = skv_p // 128

    in_maps = []
    for c in range(NCORES):
        b, hg = c // (NCORES // B), c % (NCORES // B)
        cs = slice(hg * CS, (hg + 1) * CS)
        ix = idxs[b]
        n = len(ix)
        kv_p = np.zeros((skv_p, D), np.float32)
        kv_p[:n] = key_value_input[b][ix]
        m01 = np.zeros((skv_p,), np.float32)
        m01[:n] = 1.0
        mcol = np.ascontiguousarray(m01.reshape(nkt, 128).T)  # [128, nkt]
        in_maps.append({
            "xT": np.ascontiguousarray(
                query_input[b].T.reshape(NDT, 128, SQ).transpose(1, 0, 2)
            ).astype(NP_BF16),
            "kvT": np.ascontiguousarray(
                kv_p.T.reshape(NDT, 128, skv_p).transpose(1, 0, 2)
            ).astype(NP_BF16),
            "wq": np.ascontiguousarray(
                Wq[:, cs].reshape(NDT, 128, CS).transpose(1, 0, 2)).astype(NP_BF16),
            "wk": np.ascontiguousarray(
                Wk[:, cs].reshape(NDT, 128, CS).transpose(1, 0, 2)).astype(NP_BF16),
            "wv": np.ascontiguousarray(
                Wv[:, cs].reshape(NDT, 128, CS).transpose(1, 0, 2)).astype(NP_BF16),
            "wo": np.ascontiguousarray(
                Wo[cs, :].reshape(2, 128, D).transpose(1, 0, 2)).astype(NP_BF16),
            "bqk": np.ascontiguousarray(
                np.stack([bq[cs][:128], bq[cs][128:],
                          bk[cs][:128], bk[cs][128:]], axis=1)),
            "mcol": mcol,
            "ident": np.eye(128, dtype=np.float32).astype(NP_BF16),
        })
    return in_maps


def kernel(query_input, key_value_input, key_padding_mask,
           Wq, bq, Wk, bk, Wv, bv, Wo, bo):
    query_input = np.asarray(query_input, np.float32)
    key_value_input = np.asarray(key_value_input, np.float32)
    key_padding_mask = np.asarray(key_padding_mask)
    Wq = np.asarray(Wq, np.float32); bq = np.asarray(bq, np.float32)
    Wk = np.asarray(Wk, np.float32); bk = np.asarray(bk, np.float32)
    Wv = np.asarray(Wv, np.float32); bv = np.asarray(bv, np.float32)
    Wo = np.asarray(Wo, np.float32); bo = np.asarray(bo, np.float32)

    in_maps = _shard_inputs(query_input, key_value_input, key_padding_mask,
                            Wq, bq, Wk, bk, Wv, bv, Wo, bo)
    nc = _get_nc(_SKV_P)
    res = run_bass_kernel_spmd(nc, in_maps, core_ids=list(range(NCORES)))

    # unshard: sum the 4 row-parallel partials per batch; V-bias contributes a
    # constant row (softmax rows sum to 1) folded in with bo here.
    const_row = (bv.astype(np.float64) @ Wo.astype(np.float64)) + bo.astype(np.float64)
    gpc = NCORES // B
    out = np.empty((B, SQ, D), np.float32)
    for b in range(B):
        acc = np.zeros((SQ, D), np.float64)
        for hg in range(gpc):
            acc += res.results[b * gpc + hg]["out_p"].astype(np.float64)
        out[b] = (acc + const_row[None, :]).astype(np.float32)
    return out
